# revision 31
# baseline (speedup 1.0000x reference)
"""Multi-head causal attention (B=2, N=2048, D=1024, H=16) on 8 TRN2 NeuronCores.

Sharding: data-parallel over batch (2) x tensor-parallel over head groups (4),
so each core handles one batch element and 4 heads (256 of the 1024 hidden
channels). Wq/Wk/Wv are column-sharded, Wo row-sharded; each core emits a
partial output [2048, 1024] that the host sums over the 4 head groups.

Single fully-interleaved schedule (all matmuls bf16, fp32 PSUM):
  - The QKV projections, attention (S = K^T Q -> exp -> PV), and output
    projection are woven into ONE instruction stream per engine so the
    Act-engine exp work (~70us) hides under projection matmuls and the PE
    never idles between phases. Background work (next seq-chunk's Q/K/V
    projections, previous chunk's output projection) is kept in a FIFO of
    small emission units popped between attention k-tile groups.
  - Causal q-restriction: for diagonal-crossing k-tiles only the q-range
    that can be unmasked is computed (S matmul rhs, exp, and PV are all
    restricted), and masking is a single [128,2,128] additive triangle
    (-256 on masked elements, applied to PSUM scores before the exp's 1/8
    scale) instead of full-tile 0/1 multiplies.
  - Layouts: head pairs packed per mt (even head partitions 0-63, odd
    64-127); S^T tiles [128k, 2x512q]; V stored per seq-tile with ones
    columns so the PV matmul accumulates the softmax denominator (even
    head: denom at U partition 64; odd head: denom at partition 0 with a
    zero strip keeping partitions 1-63 inert).
  - PSUM budget (8 banks): S ring 2x[128,1024], U accumulator 1x[128,1024]
    (PV lags exp by LAG k-tiles so the one U buffer is free in time),
    projection ring 2x[128,512].
  - Softmax normalization: denominator row -> reciprocal (DVE ucode, only
    legal at partition 0) -> partition-broadcast via a DRAM bounce on the
    gpsimd DMA queue (stride-0 partition APs are only legal for DRAM
    sources) -> ctx multiply. All off the PE queue.
"""

import sys

sys.path.insert(0, "/opt/trn_rl_repo")

from collections import deque

import numpy as np
import ml_dtypes

import concourse.bass as bass
import concourse.bacc as bacc
import concourse.mybir as mybir
from concourse.tile import TileContext
from concourse.bass_utils import run_bass_kernel_spmd

BF16 = mybir.dt.bfloat16
F32 = mybir.dt.float32

B, N, D, H = 2, 2048, 1024, 16
HD = 64          # head dim
HPC = 4          # heads per core
DH = HPC * HD    # 256 hidden channels per core
NCORES = 8
KT = D // 128    # 8 contraction tiles over D
ST = N // 128    # 16 seq tiles
QC = N // 512    # 4 q-chunks of 512

LAG = {0: 4, 1: 6, 2: 6, 3: 6}   # PV trails exp by this many k-tiles
POPS = {0: 4, 1: 2, 2: 2, 3: 2}  # background units popped per k-tile iter

# v_sb per-seq-tile column layout: for each head pair, an "even" block
# [V(64) | ones(1)] (matmul M=65 -> U partitions 0..64, denom at 64) and an
# "odd" block [ones(1) | zeros(63) | V(64)] (M=128 -> U partitions 64..127
# hold data, denom at partition 0, zeros keep partitions 1..63 inert).
V_BLK = {0: (0, 65), 1: (65, 193), 2: (193, 258), 3: (258, 386)}
V_COLS = 386


def _build_nc(debug: bool = False) -> bass.Bass:
    nc = bacc.Bacc()
    xT = nc.declare_dram_parameter("xT", [D, N], BF16, isOutput=False)
    wq = nc.declare_dram_parameter("wq", [D, DH], BF16, isOutput=False)
    wk = nc.declare_dram_parameter("wk", [D, DH], BF16, isOutput=False)
    wv = nc.declare_dram_parameter("wv", [D, DH], BF16, isOutput=False)
    wo = nc.declare_dram_parameter("wo", [DH, D], BF16, isOutput=False)
    y = nc.declare_dram_parameter("y", [N, D], BF16, isOutput=True)
    if debug:
        dbg = {
            "d_qT": nc.declare_dram_parameter("d_qT", [128, 2 * N], F32, isOutput=True),
            "d_kT": nc.declare_dram_parameter("d_kT", [128, 2 * N], F32, isOutput=True),
            "d_v": nc.declare_dram_parameter("d_v", [128, ST * V_COLS], F32, isOutput=True),
            "d_ctxT": nc.declare_dram_parameter("d_ctxT", [128, 2 * N], F32, isOutput=True),
        }

    xT_r = xT.rearrange("(t p) n -> t p n", p=128)
    wq_r = wq.rearrange("(t p) m -> t p m", p=128)
    wk_r = wk.rearrange("(t p) m -> t p m", p=128)
    wv_r = wv.rearrange("(t p) m -> t p m", p=128)
    wo_r = wo.rearrange("(t p) m -> t p m", p=128)
    y_r = y.rearrange("(t p) m -> t p m", p=128)

    with TileContext(nc) as tc:
        with (
            tc.tile_pool(name="const", bufs=1) as cpool,
            tc.tile_pool(name="io", bufs=3) as io_pool,
            tc.tile_pool(name="exps", bufs=8) as ex_pool,
            tc.tile_pool(name="small", bufs=4) as small_pool,
            tc.tile_pool(name="ps_s", bufs=2, space="PSUM") as ps_s_pool,
            tc.tile_pool(name="ps_u", bufs=1, space="PSUM") as ps_u_pool,
            tc.tile_pool(name="ps_p", bufs=2, space="PSUM") as ps_p_pool,
        ):
            xT_sb = cpool.tile([128, KT, N], BF16)
            wq_sb = cpool.tile([128, KT, DH], BF16)
            wk_sb = cpool.tile([128, KT, DH], BF16)
            wv_sb = cpool.tile([128, KT, DH], BF16)
            wo_sb = cpool.tile([128, 2, D], BF16)
            qT_sb = cpool.tile([128, 2, N], BF16)
            kT_sb = cpool.tile([128, 2, N], BF16)
            v_sb = cpool.tile([128, ST, V_COLS], BF16)
            # per-chunk ctx tiles: a single [128, 2, N] tile accumulates so
            # many distinct access regions that the subtile dependency
            # tracker misses write->read edges (observed as a nondeterministic
            # race: O-proj matmuls reading ctx before the normalize multiply).
            ctx_sb = [cpool.tile([128, 2, 512], BF16, name=f"ctx{c}") for c in range(QC)]
            tri_sb = cpool.tile([128, 2, 128], BF16)
            ones_sb = cpool.tile([128, 64], BF16)

            # ---- input DMAs, in consumption order ----
            # sync queue: wq + chunk-0 x columns (gate the first matmuls),
            # then wk/wv and chunk 1; Activation hwdge queue (otherwise idle):
            # chunks 2-3 and wo, halving the serial input-load time.
            xT_c = xT.rearrange("(t p) (c n) -> c p t n", p=128, n=512)
            xc_sb = xT_sb.rearrange("p t (c n) -> c p t n", n=512)
            # chunk-0 trickles in kt-slices so the first projection matmuls
            # start as soon as their own (wq[kt], xT[kt]) pair lands
            for kt in range(KT):
                nc.scalar.dma_start(out=wq_sb[:, kt, :], in_=wq_r[kt])
                nc.sync.dma_start(out=xT_sb[:, kt, 0:512], in_=xT_r[kt][:, 0:512])
            nc.scalar.dma_start(out=wk_sb, in_=wk.rearrange("(t p) m -> p t m", p=128))
            nc.scalar.dma_start(out=wv_sb, in_=wv.rearrange("(t p) m -> p t m", p=128))
            nc.sync.dma_start(out=xc_sb[1], in_=xT_c[1])
            nc.scalar.dma_start(out=xc_sb[2], in_=xT_c[2])
            nc.sync.dma_start(out=xc_sb[3], in_=xT_c[3])
            nc.scalar.dma_start(
                out=wo_sb, in_=wo.rearrange("(t p) m -> p t m", p=128)
            )

            # Causal triangle for the q-block crossing each diagonal k-tile:
            # 1.0 where q >= k (keep), 0.0 masked; multiplies exp's output so
            # the S->exp chain has no DVE hop (the exp->PV path has LAG slack).
            nc.vector.memset(tri_sb, 1.0)
            nc.gpsimd.affine_select(
                out=tri_sb,
                in_=tri_sb,
                compare_op=mybir.AluOpType.is_ge,
                fill=0.0,
                base=0,
                pattern=[[0, 2], [1, 128]],
                channel_multiplier=-1,
            )
            nc.vector.memset(ones_sb, 1.0)

            # ones / zeros scaffolding of the V blocks (all seq tiles at once)
            nc.vector.memset(v_sb[:, :, 66:129], 0.0)
            nc.vector.memset(v_sb[:, :, 259:322], 0.0)
            for col in (64, 65, 257, 258):
                nc.vector.memset(v_sb[:, :, col : col + 1], 1.0)

            # ---- emission helpers ----
            def qk_proj_half(w_sb, dst_sb, mt, c, lo_half, state):
                # half a [128, 512] projection accumulation group (4 of 8 kt)
                if lo_half:
                    state["ps"] = ps_p_pool.tile(
                        [128, 512], F32, tag="p", name="ps_qk"
                    )
                ps = state["ps"]
                for kt in range(0 if lo_half else 4, 4 if lo_half else 8):
                    nc.tensor.matmul(
                        ps,
                        lhsT=w_sb[:, kt, 128 * mt : 128 * (mt + 1)],
                        rhs=xT_sb[:, kt, 512 * c : 512 * (c + 1)],
                        start=(kt == 0),
                        stop=(kt == KT - 1),
                    )
                if not lo_half:
                    nc.vector.tensor_copy(
                        dst_sb[:, mt, 512 * c : 512 * (c + 1)], ps
                    )

            def v_proj(st):
                ps = ps_p_pool.tile([128, 512], F32, tag="p", name="ps_v")
                psv = ps[:, 0:DH]
                for kt in range(KT):
                    nc.tensor.matmul(
                        psv,
                        lhsT=xT_sb[:, kt, 128 * st : 128 * (st + 1)],
                        rhs=wv_sb[:, kt, :],
                        start=(kt == 0),
                        stop=(kt == KT - 1),
                    )
                ps_h = psv.rearrange("p (h d) -> p h d", d=HD)
                # even heads 0,2 -> v col offsets 0,193; odd heads 1,3 -> 129,322
                ev = bass.AP(
                    tensor=v_sb.tensor,
                    offset=v_sb[:, st, 0:1].offset,
                    ap=[v_sb.ap[0], [193, 2], [1, HD]],
                )
                od = bass.AP(
                    tensor=v_sb.tensor,
                    offset=v_sb[:, st, 129:130].offset,
                    ap=[v_sb.ap[0], [193, 2], [1, HD]],
                )
                in_ev = bass.AP(
                    tensor=ps.tensor,
                    offset=ps_h[:, 0, :].offset,
                    ap=[ps.ap[0], [2 * HD, 2], [1, HD]],
                )
                in_od = bass.AP(
                    tensor=ps.tensor,
                    offset=ps_h[:, 1, :].offset,
                    ap=[ps.ap[0], [2 * HD, 2], [1, HD]],
                )
                nc.vector.tensor_copy(ev, in_ev)
                nc.vector.tensor_copy(od, in_od)

            def o_proj_tile(st):
                # kt2-outer so each ctx weight tile loads once for both
                # output halves (halves the LDWEIGHTS count)
                pss = [
                    ps_p_pool.tile([128, 512], F32, tag="p", name="ps_o")
                    for _ in range(2)
                ]
                for kt2 in range(2):
                    for half in range(2):
                        nc.tensor.matmul(
                            pss[half],
                            lhsT=ctx_sb[st // 4][
                                :, kt2, 128 * (st % 4) : 128 * (st % 4 + 1)
                            ],
                            rhs=wo_sb[:, kt2, 512 * half : 512 * (half + 1)],
                            start=(kt2 == 0),
                            stop=(kt2 == 1),
                        )
                ysb = io_pool.tile([128, 1024], BF16, tag="y", name="ysb")
                for half in range(2):
                    nc.vector.tensor_copy(
                        ysb[:, 512 * half : 512 * (half + 1)], pss[half]
                    )
                nc.sync.dma_start(out=y_r[st], in_=ysb)

            units = deque()

            def add_qk(w_sb, dst_sb, mt, c):
                state = {}
                units.append(lambda: qk_proj_half(w_sb, dst_sb, mt, c, True, state))
                units.append(lambda: qk_proj_half(w_sb, dst_sb, mt, c, False, state))

            def pop_unit():
                if units:
                    units.popleft()()

            def emit_pv(uv, mt, nkt, ex, kt, qlo):
                for parity in (0, 1):
                    head = 2 * mt + parity
                    blo, bhi = V_BLK[head]
                    nc.tensor.matmul(
                        uv[0 : bhi - blo, parity, qlo:512],
                        lhsT=v_sb[:, kt, blo:bhi],
                        rhs=ex[:, parity, qlo:512],
                        start=(kt == 0),
                        stop=(kt == nkt - 1),
                        skip_group_check=True,
                    )

            def normalize_a(uv, mt, qc):
                # Part A (DVE only, emitted right at block end): pull the
                # denominator rows out of U into SBUF bf16 rows for the
                # partition-broadcast matmul. reciprocal_approx_fast (custom
                # DVE ucode) only works on APs based at partition 0, so the
                # odd head (denom at partition 0) inverts here and broadcasts
                # 1/r; the even head (denom at partition 64) broadcasts raw r
                # and inverts after the broadcast.
                rr = small_pool.tile([128, 2, 512], BF16, tag="rr", name="rr")
                nc.vector.tensor_copy(rr[64:65, 0, :], uv[64:65, 0, :])
                ri = small_pool.tile([128, 512], F32, tag="ri", name="ri")
                nc.vector.reciprocal_approx_fast(out=ri[0:1, :], in_=uv[0:1, 1, :])
                nc.vector.tensor_copy(rr[0:1, 1, :], ri[0:1, :])
                return uv, rr

            def normalize_b(state, mt, qc):
                # Part B (a couple of iters into the next block): broadcast
                # along partitions via a ones-column matmul into PSUM, invert
                # the even head's r, and scale U into ctxT. Frees U ~2us after
                # the block's last PV instead of ~7us (DRAM-bounce dance).
                uv, rr = state
                psb0 = ps_p_pool.tile([128, 512], F32, tag="p", name="psb0")
                psb1 = ps_p_pool.tile([128, 512], F32, tag="p", name="psb1")
                nc.tensor.matmul(
                    psb0[0:64, :],
                    lhsT=ones_sb[64:65, :],
                    rhs=rr[64:65, 0, :],
                    start=True,
                    stop=True,
                )
                nc.tensor.matmul(
                    psb1[64:128, :],
                    lhsT=ones_sb[0:1, :],
                    rhs=rr[0:1, 1, :],
                    start=True,
                    stop=True,
                )
                rb = small_pool.tile([128, 512], F32, tag="rb", name="rb")
                nc.vector.reciprocal_approx_fast(out=rb[0:64, :], in_=psb0[0:64, :])
                # DVE reads at most one PSUM operand per op: the ctx multiply
                # reads U from PSUM, so the odd broadcast hops through SBUF.
                nc.vector.tensor_copy(rb[64:128, :], psb1[64:128, :])
                nc.vector.tensor_mul(
                    ctx_sb[qc][0:64, mt, :],
                    uv[0:64, 0, :],
                    rb[0:64, :],
                )
                nc.vector.tensor_mul(
                    ctx_sb[qc][64:128, mt, :],
                    uv[64:128, 1, :],
                    rb[64:128, :],
                )

            # ---- prelude: chunk-0 Q/K for head-pair 0 emitted directly ----
            st0 = {}
            qk_proj_half(wq_sb, qT_sb, 0, 0, True, st0)
            qk_proj_half(wq_sb, qT_sb, 0, 0, False, st0)
            st1 = {}
            qk_proj_half(wk_sb, kT_sb, 0, 0, True, st1)
            qk_proj_half(wk_sb, kT_sb, 0, 0, False, st1)

            # background FIFO (deadlines: proj chunk c fully done before its
            # attention/PV first touches it; O-proj chunk c queued once both
            # head-pairs' ctx for chunk c are final)
            add_qk(wq_sb, qT_sb, 1, 0)
            add_qk(wk_sb, kT_sb, 1, 0)
            for st in range(4):
                units.append(lambda st=st: v_proj(st))
            for mt in range(2):
                add_qk(wq_sb, qT_sb, mt, 1)
            for mt in range(2):
                add_qk(wk_sb, kT_sb, mt, 1)

            # ---- main attention loop, background units interleaved ----
            pending_b = None
            for qc in range(QC):
                # Each chunk's proj/O units enter the pop queue at the latest
                # block where their deadline still holds, keeping the deep
                # late blocks supplied with background PE work. O(c) may only
                # enter once chunk-c ctx is final (end of qc==c) — popping it
                # earlier deadlocks the PE queue behind its own ctx read.
                if qc == 1:
                    for st in range(4, 8):
                        units.append(lambda st=st: v_proj(st))
                    for mt in range(2):
                        add_qk(wq_sb, qT_sb, mt, 2)
                if qc == 2:
                    for mt in range(2):
                        add_qk(wk_sb, kT_sb, mt, 2)
                    for st in range(8, 12):
                        units.append(lambda st=st: v_proj(st))
                    for st in range(0, 8):
                        units.append(lambda st=st: o_proj_tile(st))
                    for mt in range(2):
                        add_qk(wq_sb, qT_sb, mt, 3)
                if qc == 3:
                    for mt in range(2):
                        add_qk(wk_sb, kT_sb, mt, 3)
                    for st in range(12, 16):
                        units.append(lambda st=st: v_proj(st))
                    for st in range(8, 12):
                        units.append(lambda st=st: o_proj_tile(st))
                nkt = 4 * (qc + 1)
                for mt in range(2):
                    # U is allocated lazily at the first PV so the ring's
                    # reuse dependency is recorded AFTER the deferred
                    # normalize_b reads of the previous block's U exist —
                    # allocating at block start lets the new block's start=True
                    # PV zero the bank while normalize_b still reads it.
                    ublock = []

                    def uview():
                        if not ublock:
                            u = ps_u_pool.tile([128, 1024], F32, tag="u", name="u")
                            ublock.append(u.rearrange("p (h q) -> p h q", q=512))
                        return ublock[0]

                    pv_q = []
                    for kt in range(nkt):
                        di = kt - 4 * qc
                        qlo = 128 * di if di >= 0 else 0
                        ps_s = ps_s_pool.tile([128, 1024], F32, tag="s", name="ps_s")
                        sv = ps_s.rearrange("p (h q) -> p h q", q=512)
                        for parity in (0, 1):
                            pofs = 64 * parity
                            nc.tensor.matmul(
                                sv[:, parity, qlo:512],
                                lhsT=kT_sb[
                                    pofs : pofs + 64, mt, 128 * kt : 128 * (kt + 1)
                                ],
                                rhs=qT_sb[
                                    pofs : pofs + 64,
                                    mt,
                                    512 * qc + qlo : 512 * (qc + 1),
                                ],
                                start=True,
                                stop=True,
                            )
                        ex = ex_pool.tile([128, 2, 512], BF16, tag="ex", name="ex")
                        nc.scalar.activation(
                            ex[:, :, qlo:512],
                            sv[:, :, qlo:512],
                            mybir.ActivationFunctionType.Exp,
                            scale=1.0 / np.sqrt(HD),
                        )
                        if di >= 0:
                            # gpsimd (Pool) is idle during attention and the
                            # mask is SBUF-only -> keep it off the busy DVE
                            nc.gpsimd.tensor_mul(
                                ex[:, :, qlo : qlo + 128],
                                ex[:, :, qlo : qlo + 128],
                                tri_sb,
                            )
                        pv_q.append((ex, kt, qlo))
                        # the deferred normalize_b must precede any popped
                        # unit (an O-proj unit may read the ctx it writes),
                        # and nothing pops at kt==0 so it's never outrun
                        if kt == 1 and pending_b is not None:
                            normalize_b(*pending_b)
                            pending_b = None
                        if kt > 0:
                            for _ in range(POPS[qc]):
                                pop_unit()
                        if len(pv_q) > LAG[qc]:
                            emit_pv(uview(), mt, nkt, *pv_q.pop(0))
                    while pv_q:
                        emit_pv(uview(), mt, nkt, *pv_q.pop(0))
                    pending_b = (normalize_a(uview(), mt, qc), mt, qc)

            # ---- drain: leftover units, then chunk-3 output projection ----
            normalize_b(*pending_b)
            while units:
                pop_unit()
            for st in range(12, 16):
                o_proj_tile(st)

            if debug:
                for c in range(QC):
                    for mt in range(2):
                        tmpc = io_pool.tile([128, 1024], F32, tag="dtmp", name="dtmpc")
                        nc.vector.tensor_copy(tmpc[:, 0:512], ctx_sb[c][:, mt, :])
                        nc.sync.dma_start(
                            out=dbg["d_ctxT"][:, 2048 * mt + 512 * c : 2048 * mt + 512 * (c + 1)],
                            in_=tmpc[:, 0:512],
                        )
                for nm, sb in (
                    ("d_qT", qT_sb),
                    ("d_kT", kT_sb),
                    ("d_v", v_sb),
                ):
                    flat = sb.rearrange("p a b -> p (a b)")
                    w = flat.shape[1]
                    for off in range(0, w, 512):
                        wid = min(512, w - off)
                        tmp2 = io_pool.tile([128, 1024], F32, tag="dtmp", name="dtmp")
                        nc.vector.tensor_copy(tmp2[:, 0:wid], flat[:, off : off + wid])
                        nc.sync.dma_start(
                            out=dbg[nm][:, off : off + wid], in_=tmp2[:, 0:wid]
                        )
    nc.finalize()
    return nc


_NC = None


def _get_nc():
    global _NC
    if _NC is None:
        _NC = _build_nc()
    return _NC


def kernel(x, Wq, Wk, Wv, Wo):
    x = np.asarray(x, dtype=np.float32)
    bf = ml_dtypes.bfloat16
    in_maps = []
    for c in range(NCORES):
        b, g = divmod(c, 4)
        sl = slice(g * DH, (g + 1) * DH)
        in_maps.append(
            {
                "xT": np.ascontiguousarray(x[b].T).astype(bf),
                "wq": np.ascontiguousarray(np.asarray(Wq)[:, sl]).astype(bf),
                "wk": np.ascontiguousarray(np.asarray(Wk)[:, sl]).astype(bf),
                "wv": np.ascontiguousarray(np.asarray(Wv)[:, sl]).astype(bf),
                "wo": np.ascontiguousarray(np.asarray(Wo)[sl, :]).astype(bf),
            }
        )
    global _last_in_maps
    _last_in_maps = in_maps
    res = run_bass_kernel_spmd(
        _get_nc(), in_maps, core_ids=list(range(NCORES)), trace=False
    )
    out = np.zeros((B, N, D), dtype=np.float32)
    for c in range(NCORES):
        out[c // 4] += res.results[c]["y"]
    return out


# revision 32
# speedup vs baseline: 1.0119x; 1.0119x over previous
"""Multi-head causal attention (B=2, N=2048, D=1024, H=16) on 8 TRN2 NeuronCores.

Sharding: data-parallel over batch (2) x tensor-parallel over head groups (4),
so each core handles one batch element and 4 heads (256 of the 1024 hidden
channels). Wq/Wk/Wv are column-sharded, Wo row-sharded; each core emits a
partial output [2048, 1024] that the host sums over the 4 head groups.

Single fully-interleaved schedule (all matmuls bf16, fp32 PSUM):
  - The QKV projections, attention (S = K^T Q -> exp -> PV), and output
    projection are woven into ONE instruction stream per engine so the
    Act-engine exp work (~70us) hides under projection matmuls and the PE
    never idles between phases. Background work (next seq-chunk's Q/K/V
    projections, previous chunk's output projection) is kept in a FIFO of
    small emission units popped between attention k-tile groups.
  - Causal q-restriction: for diagonal-crossing k-tiles only the q-range
    that can be unmasked is computed (S matmul rhs, exp, and PV are all
    restricted), and masking is a single [128,2,128] additive triangle
    (-256 on masked elements, applied to PSUM scores before the exp's 1/8
    scale) instead of full-tile 0/1 multiplies.
  - Layouts: head pairs packed per mt (even head partitions 0-63, odd
    64-127); S^T tiles [128k, 2x512q]; V stored per seq-tile with ones
    columns so the PV matmul accumulates the softmax denominator (even
    head: denom at U partition 64; odd head: denom at partition 0 with a
    zero strip keeping partitions 1-63 inert).
  - PSUM budget (8 banks): S ring 2x[128,1024], U accumulator 1x[128,1024]
    (PV lags exp by LAG k-tiles so the one U buffer is free in time),
    projection ring 2x[128,512].
  - Softmax normalization: denominator row -> reciprocal (DVE ucode, only
    legal at partition 0) -> partition-broadcast via a DRAM bounce on the
    gpsimd DMA queue (stride-0 partition APs are only legal for DRAM
    sources) -> ctx multiply. All off the PE queue.
"""

import sys

sys.path.insert(0, "/opt/trn_rl_repo")

from collections import deque

import numpy as np
import ml_dtypes

import concourse.bass as bass
import concourse.bacc as bacc
import concourse.mybir as mybir
from concourse.tile import TileContext
from concourse.bass_utils import run_bass_kernel_spmd

BF16 = mybir.dt.bfloat16
F32 = mybir.dt.float32

B, N, D, H = 2, 2048, 1024, 16
HD = 64          # head dim
HPC = 4          # heads per core
DH = HPC * HD    # 256 hidden channels per core
NCORES = 8
KT = D // 128    # 8 contraction tiles over D
ST = N // 128    # 16 seq tiles
QC = N // 512    # 4 q-chunks of 512

LAG = {0: 4, 1: 4, 2: 6, 3: 6}   # PV trails exp by this many k-tiles
POPS = {0: 4, 1: 2, 2: 2, 3: 2}  # background units popped per k-tile iter

# v_sb per-seq-tile column layout: for each head pair, an "even" block
# [V(64) | ones(1)] (matmul M=65 -> U partitions 0..64, denom at 64) and an
# "odd" block [ones(1) | zeros(63) | V(64)] (M=128 -> U partitions 64..127
# hold data, denom at partition 0, zeros keep partitions 1..63 inert).
V_BLK = {0: (0, 65), 1: (65, 193), 2: (193, 258), 3: (258, 386)}
V_COLS = 386


def _build_nc(debug: bool = False) -> bass.Bass:
    nc = bacc.Bacc()
    xT = nc.declare_dram_parameter("xT", [D, N], BF16, isOutput=False)
    wq = nc.declare_dram_parameter("wq", [D, DH], BF16, isOutput=False)
    wk = nc.declare_dram_parameter("wk", [D, DH], BF16, isOutput=False)
    wv = nc.declare_dram_parameter("wv", [D, DH], BF16, isOutput=False)
    wo = nc.declare_dram_parameter("wo", [DH, D], BF16, isOutput=False)
    y = nc.declare_dram_parameter("y", [N, D], BF16, isOutput=True)
    if debug:
        dbg = {
            "d_qT": nc.declare_dram_parameter("d_qT", [128, 2 * N], F32, isOutput=True),
            "d_kT": nc.declare_dram_parameter("d_kT", [128, 2 * N], F32, isOutput=True),
            "d_v": nc.declare_dram_parameter("d_v", [128, ST * V_COLS], F32, isOutput=True),
            "d_ctxT": nc.declare_dram_parameter("d_ctxT", [128, 2 * N], F32, isOutput=True),
        }

    xT_r = xT.rearrange("(t p) n -> t p n", p=128)
    wq_r = wq.rearrange("(t p) m -> t p m", p=128)
    wk_r = wk.rearrange("(t p) m -> t p m", p=128)
    wv_r = wv.rearrange("(t p) m -> t p m", p=128)
    wo_r = wo.rearrange("(t p) m -> t p m", p=128)
    y_r = y.rearrange("(t p) m -> t p m", p=128)

    with TileContext(nc) as tc:
        with (
            tc.tile_pool(name="const", bufs=1) as cpool,
            tc.tile_pool(name="io", bufs=3) as io_pool,
            tc.tile_pool(name="exps", bufs=8) as ex_pool,
            tc.tile_pool(name="small", bufs=4) as small_pool,
            tc.tile_pool(name="ps_s", bufs=2, space="PSUM") as ps_s_pool,
            tc.tile_pool(name="ps_u", bufs=1, space="PSUM") as ps_u_pool,
            tc.tile_pool(name="ps_p", bufs=2, space="PSUM") as ps_p_pool,
        ):
            xT_sb = cpool.tile([128, KT, N], BF16)
            wq_sb = cpool.tile([128, KT, DH], BF16)
            wk_sb = cpool.tile([128, KT, DH], BF16)
            wv_sb = cpool.tile([128, KT, DH], BF16)
            wo_sb = cpool.tile([128, 2, D], BF16)
            qT_sb = cpool.tile([128, 2, N], BF16)
            kT_sb = cpool.tile([128, 2, N], BF16)
            v_sb = cpool.tile([128, ST, V_COLS], BF16)
            # per-chunk ctx tiles: a single [128, 2, N] tile accumulates so
            # many distinct access regions that the subtile dependency
            # tracker misses write->read edges (observed as a nondeterministic
            # race: O-proj matmuls reading ctx before the normalize multiply).
            ctx_sb = [cpool.tile([128, 2, 512], BF16, name=f"ctx{c}") for c in range(QC)]
            tri_sb = cpool.tile([128, 2, 128], BF16)
            ones_sb = cpool.tile([128, 64], BF16)

            # ---- input DMAs, in consumption order ----
            # sync queue: wq + chunk-0 x columns (gate the first matmuls),
            # then wk/wv and chunk 1; Activation hwdge queue (otherwise idle):
            # chunks 2-3 and wo, halving the serial input-load time.
            xT_c = xT.rearrange("(t p) (c n) -> c p t n", p=128, n=512)
            xc_sb = xT_sb.rearrange("p t (c n) -> c p t n", n=512)
            # chunk-0 trickles in kt-slices so the first projection matmuls
            # start as soon as their own (wq[kt], xT[kt]) pair lands
            for kt in range(KT):
                nc.scalar.dma_start(out=wq_sb[:, kt, :], in_=wq_r[kt])
                nc.sync.dma_start(out=xT_sb[:, kt, 0:512], in_=xT_r[kt][:, 0:512])
            nc.scalar.dma_start(out=wk_sb, in_=wk.rearrange("(t p) m -> p t m", p=128))
            nc.scalar.dma_start(out=wv_sb, in_=wv.rearrange("(t p) m -> p t m", p=128))
            nc.sync.dma_start(out=xc_sb[1], in_=xT_c[1])
            nc.scalar.dma_start(out=xc_sb[2], in_=xT_c[2])
            nc.sync.dma_start(out=xc_sb[3], in_=xT_c[3])
            nc.scalar.dma_start(
                out=wo_sb, in_=wo.rearrange("(t p) m -> p t m", p=128)
            )

            # Causal triangle for the q-block crossing each diagonal k-tile:
            # 1.0 where q >= k (keep), 0.0 masked; multiplies exp's output so
            # the S->exp chain has no DVE hop (the exp->PV path has LAG slack).
            nc.vector.memset(tri_sb, 1.0)
            nc.gpsimd.affine_select(
                out=tri_sb,
                in_=tri_sb,
                compare_op=mybir.AluOpType.is_ge,
                fill=0.0,
                base=0,
                pattern=[[0, 2], [1, 128]],
                channel_multiplier=-1,
            )
            nc.vector.memset(ones_sb, 1.0)

            # ones / zeros scaffolding of the V blocks (all seq tiles at once)
            nc.vector.memset(v_sb[:, :, 66:129], 0.0)
            nc.vector.memset(v_sb[:, :, 259:322], 0.0)
            for col in (64, 65, 257, 258):
                nc.vector.memset(v_sb[:, :, col : col + 1], 1.0)

            # ---- emission helpers ----
            def qk_proj_half(w_sb, dst_sb, mt, c, lo_half, state):
                # half a [128, 512] projection accumulation group (4 of 8 kt)
                if lo_half:
                    state["ps"] = ps_p_pool.tile(
                        [128, 512], F32, tag="p", name="ps_qk"
                    )
                ps = state["ps"]
                for kt in range(0 if lo_half else 4, 4 if lo_half else 8):
                    nc.tensor.matmul(
                        ps,
                        lhsT=w_sb[:, kt, 128 * mt : 128 * (mt + 1)],
                        rhs=xT_sb[:, kt, 512 * c : 512 * (c + 1)],
                        start=(kt == 0),
                        stop=(kt == KT - 1),
                    )
                if not lo_half:
                    nc.vector.tensor_copy(
                        dst_sb[:, mt, 512 * c : 512 * (c + 1)], ps
                    )

            def v_proj(st):
                ps = ps_p_pool.tile([128, 512], F32, tag="p", name="ps_v")
                psv = ps[:, 0:DH]
                for kt in range(KT):
                    nc.tensor.matmul(
                        psv,
                        lhsT=xT_sb[:, kt, 128 * st : 128 * (st + 1)],
                        rhs=wv_sb[:, kt, :],
                        start=(kt == 0),
                        stop=(kt == KT - 1),
                    )
                ps_h = psv.rearrange("p (h d) -> p h d", d=HD)
                # even heads 0,2 -> v col offsets 0,193; odd heads 1,3 -> 129,322
                ev = bass.AP(
                    tensor=v_sb.tensor,
                    offset=v_sb[:, st, 0:1].offset,
                    ap=[v_sb.ap[0], [193, 2], [1, HD]],
                )
                od = bass.AP(
                    tensor=v_sb.tensor,
                    offset=v_sb[:, st, 129:130].offset,
                    ap=[v_sb.ap[0], [193, 2], [1, HD]],
                )
                in_ev = bass.AP(
                    tensor=ps.tensor,
                    offset=ps_h[:, 0, :].offset,
                    ap=[ps.ap[0], [2 * HD, 2], [1, HD]],
                )
                in_od = bass.AP(
                    tensor=ps.tensor,
                    offset=ps_h[:, 1, :].offset,
                    ap=[ps.ap[0], [2 * HD, 2], [1, HD]],
                )
                nc.vector.tensor_copy(ev, in_ev)
                nc.vector.tensor_copy(od, in_od)

            def o_proj_tile(st):
                # kt2-outer so each ctx weight tile loads once for both
                # output halves (halves the LDWEIGHTS count)
                pss = [
                    ps_p_pool.tile([128, 512], F32, tag="p", name="ps_o")
                    for _ in range(2)
                ]
                for kt2 in range(2):
                    for half in range(2):
                        nc.tensor.matmul(
                            pss[half],
                            lhsT=ctx_sb[st // 4][
                                :, kt2, 128 * (st % 4) : 128 * (st % 4 + 1)
                            ],
                            rhs=wo_sb[:, kt2, 512 * half : 512 * (half + 1)],
                            start=(kt2 == 0),
                            stop=(kt2 == 1),
                        )
                ysb = io_pool.tile([128, 1024], BF16, tag="y", name="ysb")
                for half in range(2):
                    nc.vector.tensor_copy(
                        ysb[:, 512 * half : 512 * (half + 1)], pss[half]
                    )
                nc.sync.dma_start(out=y_r[st], in_=ysb)

            units = deque()

            def add_qk(w_sb, dst_sb, mt, c):
                state = {}
                units.append(lambda: qk_proj_half(w_sb, dst_sb, mt, c, True, state))
                units.append(lambda: qk_proj_half(w_sb, dst_sb, mt, c, False, state))

            def pop_unit():
                if units:
                    units.popleft()()

            def emit_pv(uv, mt, nkt, ex, kt, qlo):
                for parity in (0, 1):
                    head = 2 * mt + parity
                    blo, bhi = V_BLK[head]
                    nc.tensor.matmul(
                        uv[0 : bhi - blo, parity, qlo:512],
                        lhsT=v_sb[:, kt, blo:bhi],
                        rhs=ex[:, parity, qlo:512],
                        start=(kt == 0),
                        stop=(kt == nkt - 1),
                        skip_group_check=True,
                    )

            def normalize_a(uv, mt, qc):
                # Part A (DVE only, emitted right at block end): pull the
                # denominator rows out of U into SBUF bf16 rows for the
                # partition-broadcast matmul. reciprocal_approx_fast (custom
                # DVE ucode) only works on APs based at partition 0, so the
                # odd head (denom at partition 0) inverts here and broadcasts
                # 1/r; the even head (denom at partition 64) broadcasts raw r
                # and inverts after the broadcast.
                rr = small_pool.tile([128, 2, 512], BF16, tag="rr", name="rr")
                nc.vector.tensor_copy(rr[64:65, 0, :], uv[64:65, 0, :])
                ri = small_pool.tile([128, 512], F32, tag="ri", name="ri")
                nc.vector.reciprocal_approx_fast(out=ri[0:1, :], in_=uv[0:1, 1, :])
                nc.vector.tensor_copy(rr[0:1, 1, :], ri[0:1, :])
                return uv, rr

            def normalize_b(state, mt, qc):
                # Part B (a couple of iters into the next block): broadcast
                # along partitions via a ones-column matmul into PSUM, invert
                # the even head's r, and scale U into ctxT. Frees U ~2us after
                # the block's last PV instead of ~7us (DRAM-bounce dance).
                uv, rr = state
                psb0 = ps_p_pool.tile([128, 512], F32, tag="p", name="psb0")
                psb1 = ps_p_pool.tile([128, 512], F32, tag="p", name="psb1")
                nc.tensor.matmul(
                    psb0[0:64, :],
                    lhsT=ones_sb[64:65, :],
                    rhs=rr[64:65, 0, :],
                    start=True,
                    stop=True,
                )
                nc.tensor.matmul(
                    psb1[64:128, :],
                    lhsT=ones_sb[0:1, :],
                    rhs=rr[0:1, 1, :],
                    start=True,
                    stop=True,
                )
                rb = small_pool.tile([128, 512], F32, tag="rb", name="rb")
                nc.vector.reciprocal_approx_fast(out=rb[0:64, :], in_=psb0[0:64, :])
                # DVE reads at most one PSUM operand per op: the ctx multiply
                # reads U from PSUM, so the odd broadcast hops through SBUF.
                nc.vector.tensor_copy(rb[64:128, :], psb1[64:128, :])
                nc.vector.tensor_mul(
                    ctx_sb[qc][0:64, mt, :],
                    uv[0:64, 0, :],
                    rb[0:64, :],
                )
                nc.vector.tensor_mul(
                    ctx_sb[qc][64:128, mt, :],
                    uv[64:128, 1, :],
                    rb[64:128, :],
                )

            # ---- prelude: chunk-0 Q/K for head-pair 0 emitted directly ----
            st0 = {}
            qk_proj_half(wq_sb, qT_sb, 0, 0, True, st0)
            qk_proj_half(wq_sb, qT_sb, 0, 0, False, st0)
            st1 = {}
            qk_proj_half(wk_sb, kT_sb, 0, 0, True, st1)
            qk_proj_half(wk_sb, kT_sb, 0, 0, False, st1)

            # background FIFO (deadlines: proj chunk c fully done before its
            # attention/PV first touches it; O-proj chunk c queued once both
            # head-pairs' ctx for chunk c are final)
            add_qk(wq_sb, qT_sb, 1, 0)
            add_qk(wk_sb, kT_sb, 1, 0)
            for st in range(4):
                units.append(lambda st=st: v_proj(st))
            for mt in range(2):
                add_qk(wq_sb, qT_sb, mt, 1)
            for mt in range(2):
                add_qk(wk_sb, kT_sb, mt, 1)

            # ---- main attention loop, background units interleaved ----
            pending_b = None
            for qc in range(QC):
                # Each chunk's proj/O units enter the pop queue at the latest
                # block where their deadline still holds, keeping the deep
                # late blocks supplied with background PE work. O(c) may only
                # enter once chunk-c ctx is final (end of qc==c) — popping it
                # earlier deadlocks the PE queue behind its own ctx read.
                if qc == 1:
                    for st in range(4, 8):
                        units.append(lambda st=st: v_proj(st))
                    for mt in range(2):
                        add_qk(wq_sb, qT_sb, mt, 2)
                if qc == 2:
                    for mt in range(2):
                        add_qk(wk_sb, kT_sb, mt, 2)
                    for st in range(8, 12):
                        units.append(lambda st=st: v_proj(st))
                    for st in range(0, 8):
                        units.append(lambda st=st: o_proj_tile(st))
                    for mt in range(2):
                        add_qk(wq_sb, qT_sb, mt, 3)
                if qc == 3:
                    for mt in range(2):
                        add_qk(wk_sb, kT_sb, mt, 3)
                    for st in range(12, 16):
                        units.append(lambda st=st: v_proj(st))
                    for st in range(8, 12):
                        units.append(lambda st=st: o_proj_tile(st))
                nkt = 4 * (qc + 1)
                for mt in range(2):
                    # U is allocated lazily at the first PV so the ring's
                    # reuse dependency is recorded AFTER the deferred
                    # normalize_b reads of the previous block's U exist —
                    # allocating at block start lets the new block's start=True
                    # PV zero the bank while normalize_b still reads it.
                    ublock = []

                    def uview():
                        if not ublock:
                            u = ps_u_pool.tile([128, 1024], F32, tag="u", name="u")
                            ublock.append(u.rearrange("p (h q) -> p h q", q=512))
                        return ublock[0]

                    pv_q = []
                    for kt in range(nkt):
                        di = kt - 4 * qc
                        qlo = 128 * di if di >= 0 else 0
                        ps_s = ps_s_pool.tile([128, 1024], F32, tag="s", name="ps_s")
                        sv = ps_s.rearrange("p (h q) -> p h q", q=512)
                        for parity in (0, 1):
                            pofs = 64 * parity
                            nc.tensor.matmul(
                                sv[:, parity, qlo:512],
                                lhsT=kT_sb[
                                    pofs : pofs + 64, mt, 128 * kt : 128 * (kt + 1)
                                ],
                                rhs=qT_sb[
                                    pofs : pofs + 64,
                                    mt,
                                    512 * qc + qlo : 512 * (qc + 1),
                                ],
                                start=True,
                                stop=True,
                            )
                        ex = ex_pool.tile([128, 2, 512], BF16, tag="ex", name="ex")
                        nc.scalar.activation(
                            ex[:, :, qlo:512],
                            sv[:, :, qlo:512],
                            mybir.ActivationFunctionType.Exp,
                            scale=1.0 / np.sqrt(HD),
                        )
                        if di >= 0:
                            # gpsimd (Pool) is idle during attention and the
                            # mask is SBUF-only -> keep it off the busy DVE
                            nc.gpsimd.tensor_mul(
                                ex[:, :, qlo : qlo + 128],
                                ex[:, :, qlo : qlo + 128],
                                tri_sb,
                            )
                        pv_q.append((ex, kt, qlo))
                        # the deferred normalize_b must precede any popped
                        # unit (an O-proj unit may read the ctx it writes),
                        # and nothing pops at kt==0 so it's never outrun
                        if kt == 1 and pending_b is not None:
                            normalize_b(*pending_b)
                            pending_b = None
                        if kt > 0:
                            for _ in range(POPS[qc]):
                                pop_unit()
                        if len(pv_q) > LAG[qc]:
                            emit_pv(uview(), mt, nkt, *pv_q.pop(0))
                    while pv_q:
                        emit_pv(uview(), mt, nkt, *pv_q.pop(0))
                    pending_b = (normalize_a(uview(), mt, qc), mt, qc)

            # ---- drain: leftover units, then chunk-3 output projection ----
            normalize_b(*pending_b)
            while units:
                pop_unit()
            for st in range(12, 16):
                o_proj_tile(st)

            if debug:
                for c in range(QC):
                    for mt in range(2):
                        tmpc = io_pool.tile([128, 1024], F32, tag="dtmp", name="dtmpc")
                        nc.vector.tensor_copy(tmpc[:, 0:512], ctx_sb[c][:, mt, :])
                        nc.sync.dma_start(
                            out=dbg["d_ctxT"][:, 2048 * mt + 512 * c : 2048 * mt + 512 * (c + 1)],
                            in_=tmpc[:, 0:512],
                        )
                for nm, sb in (
                    ("d_qT", qT_sb),
                    ("d_kT", kT_sb),
                    ("d_v", v_sb),
                ):
                    flat = sb.rearrange("p a b -> p (a b)")
                    w = flat.shape[1]
                    for off in range(0, w, 512):
                        wid = min(512, w - off)
                        tmp2 = io_pool.tile([128, 1024], F32, tag="dtmp", name="dtmp")
                        nc.vector.tensor_copy(tmp2[:, 0:wid], flat[:, off : off + wid])
                        nc.sync.dma_start(
                            out=dbg[nm][:, off : off + wid], in_=tmp2[:, 0:wid]
                        )
    nc.finalize()
    return nc


_NC = None


def _get_nc():
    global _NC
    if _NC is None:
        _NC = _build_nc()
    return _NC


def kernel(x, Wq, Wk, Wv, Wo):
    x = np.asarray(x, dtype=np.float32)
    bf = ml_dtypes.bfloat16
    in_maps = []
    for c in range(NCORES):
        b, g = divmod(c, 4)
        sl = slice(g * DH, (g + 1) * DH)
        in_maps.append(
            {
                "xT": np.ascontiguousarray(x[b].T).astype(bf),
                "wq": np.ascontiguousarray(np.asarray(Wq)[:, sl]).astype(bf),
                "wk": np.ascontiguousarray(np.asarray(Wk)[:, sl]).astype(bf),
                "wv": np.ascontiguousarray(np.asarray(Wv)[:, sl]).astype(bf),
                "wo": np.ascontiguousarray(np.asarray(Wo)[sl, :]).astype(bf),
            }
        )
    global _last_in_maps
    _last_in_maps = in_maps
    res = run_bass_kernel_spmd(
        _get_nc(), in_maps, core_ids=list(range(NCORES)), trace=False
    )
    out = np.zeros((B, N, D), dtype=np.float32)
    for c in range(NCORES):
        out[c // 4] += res.results[c]["y"]
    return out


# revision 33
# speedup vs baseline: 1.2154x; 1.2011x over previous
"""Multi-head causal attention (B=2, N=2048, D=1024, H=16) on 8 TRN2 NeuronCores.

Sharding: data-parallel over batch (2) x tensor-parallel over head groups (4),
so each core handles one batch element and 4 heads (256 of the 1024 hidden
channels). Wq/Wk/Wv are column-sharded, Wo row-sharded; each core emits a
partial output [2048, 1024] that the host sums over the 4 head groups.

Single fully-interleaved schedule (all matmuls bf16, fp32 PSUM):
  - The QKV projections, attention (S = K^T Q -> exp -> PV), and output
    projection are woven into ONE instruction stream per engine so the
    Act-engine exp work (~70us) hides under projection matmuls and the PE
    never idles between phases. Background work (next seq-chunk's Q/K/V
    projections, previous chunk's output projection) is kept in a FIFO of
    small emission units popped between attention k-tile groups.
  - Causal q-restriction: for diagonal-crossing k-tiles only the q-range
    that can be unmasked is computed (S matmul rhs, exp, and PV are all
    restricted), and masking is a single [128,2,128] additive triangle
    (-256 on masked elements, applied to PSUM scores before the exp's 1/8
    scale) instead of full-tile 0/1 multiplies.
  - Layouts: head pairs packed per mt (even head partitions 0-63, odd
    64-127); S^T tiles [128k, 2x512q]; V stored per seq-tile with ones
    columns so the PV matmul accumulates the softmax denominator (even
    head: denom at U partition 64; odd head: denom at partition 0 with a
    zero strip keeping partitions 1-63 inert).
  - PSUM budget (8 banks): S ring 2x[128,1024], U accumulator 1x[128,1024]
    (PV lags exp by LAG k-tiles so the one U buffer is free in time),
    projection ring 2x[128,512].
  - Softmax normalization: denominator row -> reciprocal (DVE ucode, only
    legal at partition 0) -> partition-broadcast via a DRAM bounce on the
    gpsimd DMA queue (stride-0 partition APs are only legal for DRAM
    sources) -> ctx multiply. All off the PE queue.
"""

import sys

sys.path.insert(0, "/opt/trn_rl_repo")

from collections import deque

import numpy as np
import ml_dtypes

import concourse.bass as bass
import concourse.bacc as bacc
import concourse.mybir as mybir
from concourse.tile import TileContext
from concourse.bass_utils import run_bass_kernel_spmd

BF16 = mybir.dt.bfloat16
F32 = mybir.dt.float32

B, N, D, H = 2, 2048, 1024, 16
HD = 64          # head dim
HPC = 4          # heads per core
DH = HPC * HD    # 256 hidden channels per core
NCORES = 8
KT = D // 128    # 8 contraction tiles over D
ST = N // 128    # 16 seq tiles
QC = N // 512    # 4 q-chunks of 512

LAG = {0: 4, 1: 4, 2: 6, 3: 6}   # PV trails exp by this many k-tiles
POPS = {0: 4, 1: 2, 2: 2, 3: 2}  # background units popped per k-tile iter

# v_sb per-seq-tile column layout: for each head pair, an "even" block
# [V(64) | ones(1)] (matmul M=65 -> U partitions 0..64, denom at 64) and an
# "odd" block [ones(1) | zeros(63) | V(64)] (M=128 -> U partitions 64..127
# hold data, denom at partition 0, zeros keep partitions 1..63 inert).
V_BLK = {0: (0, 65), 1: (65, 193), 2: (193, 258), 3: (258, 386)}
V_COLS = 386


def _build_nc(debug: bool = False) -> bass.Bass:
    nc = bacc.Bacc()
    xT = nc.declare_dram_parameter("xT", [D, N], BF16, isOutput=False)
    wq = nc.declare_dram_parameter("wq", [D, DH], BF16, isOutput=False)
    wk = nc.declare_dram_parameter("wk", [D, DH], BF16, isOutput=False)
    wv = nc.declare_dram_parameter("wv", [D, DH], BF16, isOutput=False)
    wo = nc.declare_dram_parameter("wo", [DH, D], BF16, isOutput=False)
    y = nc.declare_dram_parameter("y", [N, D], BF16, isOutput=True)
    if debug:
        dbg = {
            "d_qT": nc.declare_dram_parameter("d_qT", [128, 2 * N], F32, isOutput=True),
            "d_kT": nc.declare_dram_parameter("d_kT", [128, 2 * N], F32, isOutput=True),
            "d_v": nc.declare_dram_parameter("d_v", [128, ST * V_COLS], F32, isOutput=True),
            "d_ctxT": nc.declare_dram_parameter("d_ctxT", [128, 2 * N], F32, isOutput=True),
        }

    xT_r = xT.rearrange("(t p) n -> t p n", p=128)
    wq_r = wq.rearrange("(t p) m -> t p m", p=128)
    wk_r = wk.rearrange("(t p) m -> t p m", p=128)
    wv_r = wv.rearrange("(t p) m -> t p m", p=128)
    wo_r = wo.rearrange("(t p) m -> t p m", p=128)
    y_r = y.rearrange("(t p) m -> t p m", p=128)

    with TileContext(nc) as tc:
        with (
            tc.tile_pool(name="const", bufs=1) as cpool,
            tc.tile_pool(name="io", bufs=3) as io_pool,
            tc.tile_pool(name="exps", bufs=8) as ex_pool,
            tc.tile_pool(name="small", bufs=4) as small_pool,
            tc.tile_pool(name="ps_s", bufs=2, space="PSUM") as ps_s_pool,
            tc.tile_pool(name="ps_u", bufs=1, space="PSUM") as ps_u_pool,
            tc.tile_pool(name="ps_p", bufs=2, space="PSUM") as ps_p_pool,
        ):
            xT_sb = cpool.tile([128, KT, N], BF16)
            wq_sb = cpool.tile([128, KT, DH], BF16)
            wk_sb = cpool.tile([128, KT, DH], BF16)
            wv_sb = cpool.tile([128, KT, DH], BF16)
            wo_sb = cpool.tile([128, 2, D], BF16)
            qT_sb = cpool.tile([128, 2, N], BF16)
            kT_sb = cpool.tile([128, 2, N], BF16)
            v_sb = cpool.tile([128, ST, V_COLS], BF16)
            # per-chunk ctx tiles: a single [128, 2, N] tile accumulates so
            # many distinct access regions that the subtile dependency
            # tracker misses write->read edges (observed as a nondeterministic
            # race: O-proj matmuls reading ctx before the normalize multiply).
            ctx_sb = [cpool.tile([128, 2, 512], BF16, name=f"ctx{c}") for c in range(QC)]
            tri_sb = cpool.tile([128, 2, 128], BF16)
            ones_sb = cpool.tile([128, 64], BF16)

            # ---- input DMAs, in consumption order ----
            # sync queue: wq + chunk-0 x columns (gate the first matmuls),
            # then wk/wv and chunk 1; Activation hwdge queue (otherwise idle):
            # chunks 2-3 and wo, halving the serial input-load time.
            xT_c = xT.rearrange("(t p) (c n) -> c p t n", p=128, n=512)
            xc_sb = xT_sb.rearrange("p t (c n) -> c p t n", n=512)
            nc.scalar.dma_start(out=wq_sb, in_=wq.rearrange("(t p) m -> p t m", p=128))
            nc.sync.dma_start(out=xc_sb[0], in_=xT_c[0])
            nc.scalar.dma_start(out=wk_sb, in_=wk.rearrange("(t p) m -> p t m", p=128))
            nc.scalar.dma_start(out=wv_sb, in_=wv.rearrange("(t p) m -> p t m", p=128))
            nc.sync.dma_start(out=xc_sb[1], in_=xT_c[1])
            nc.scalar.dma_start(out=xc_sb[2], in_=xT_c[2])
            nc.sync.dma_start(out=xc_sb[3], in_=xT_c[3])
            nc.scalar.dma_start(
                out=wo_sb, in_=wo.rearrange("(t p) m -> p t m", p=128)
            )

            # Causal triangle for the q-block crossing each diagonal k-tile:
            # 1.0 where q >= k (keep), 0.0 masked; multiplies exp's output so
            # the S->exp chain has no DVE hop (the exp->PV path has LAG slack).
            nc.vector.memset(tri_sb, 1.0)
            nc.gpsimd.affine_select(
                out=tri_sb,
                in_=tri_sb,
                compare_op=mybir.AluOpType.is_ge,
                fill=0.0,
                base=0,
                pattern=[[0, 2], [1, 128]],
                channel_multiplier=-1,
            )
            nc.vector.memset(ones_sb, 1.0)

            # ones / zeros scaffolding of the V blocks (all seq tiles at once)
            nc.vector.memset(v_sb[:, :, 66:129], 0.0)
            nc.vector.memset(v_sb[:, :, 259:322], 0.0)
            for col in (64, 65, 257, 258):
                nc.vector.memset(v_sb[:, :, col : col + 1], 1.0)

            # ---- emission helpers ----
            def qk_proj_half(w_sb, dst_sb, mt, c, lo_half, state):
                # half a [128, 512] projection accumulation group (4 of 8 kt)
                if lo_half:
                    state["ps"] = ps_p_pool.tile(
                        [128, 512], F32, tag="p", name="ps_qk"
                    )
                ps = state["ps"]
                for kt in range(0 if lo_half else 4, 4 if lo_half else 8):
                    nc.tensor.matmul(
                        ps,
                        lhsT=w_sb[:, kt, 128 * mt : 128 * (mt + 1)],
                        rhs=xT_sb[:, kt, 512 * c : 512 * (c + 1)],
                        start=(kt == 0),
                        stop=(kt == KT - 1),
                    )
                if not lo_half:
                    nc.vector.tensor_copy(
                        dst_sb[:, mt, 512 * c : 512 * (c + 1)], ps
                    )

            def v_proj(st):
                ps = ps_p_pool.tile([128, 512], F32, tag="p", name="ps_v")
                psv = ps[:, 0:DH]
                for kt in range(KT):
                    nc.tensor.matmul(
                        psv,
                        lhsT=xT_sb[:, kt, 128 * st : 128 * (st + 1)],
                        rhs=wv_sb[:, kt, :],
                        start=(kt == 0),
                        stop=(kt == KT - 1),
                    )
                ps_h = psv.rearrange("p (h d) -> p h d", d=HD)
                # even heads 0,2 -> v col offsets 0,193; odd heads 1,3 -> 129,322
                ev = bass.AP(
                    tensor=v_sb.tensor,
                    offset=v_sb[:, st, 0:1].offset,
                    ap=[v_sb.ap[0], [193, 2], [1, HD]],
                )
                od = bass.AP(
                    tensor=v_sb.tensor,
                    offset=v_sb[:, st, 129:130].offset,
                    ap=[v_sb.ap[0], [193, 2], [1, HD]],
                )
                in_ev = bass.AP(
                    tensor=ps.tensor,
                    offset=ps_h[:, 0, :].offset,
                    ap=[ps.ap[0], [2 * HD, 2], [1, HD]],
                )
                in_od = bass.AP(
                    tensor=ps.tensor,
                    offset=ps_h[:, 1, :].offset,
                    ap=[ps.ap[0], [2 * HD, 2], [1, HD]],
                )
                nc.vector.tensor_copy(ev, in_ev)
                nc.vector.tensor_copy(od, in_od)

            def o_proj_tile(st):
                # kt2-outer so each ctx weight tile loads once for both
                # output halves (halves the LDWEIGHTS count)
                pss = [
                    ps_p_pool.tile([128, 512], F32, tag="p", name="ps_o")
                    for _ in range(2)
                ]
                for kt2 in range(2):
                    for half in range(2):
                        nc.tensor.matmul(
                            pss[half],
                            lhsT=ctx_sb[st // 4][
                                :, kt2, 128 * (st % 4) : 128 * (st % 4 + 1)
                            ],
                            rhs=wo_sb[:, kt2, 512 * half : 512 * (half + 1)],
                            start=(kt2 == 0),
                            stop=(kt2 == 1),
                        )
                ysb = io_pool.tile([128, 1024], BF16, tag="y", name="ysb")
                for half in range(2):
                    nc.vector.tensor_copy(
                        ysb[:, 512 * half : 512 * (half + 1)], pss[half]
                    )
                nc.sync.dma_start(out=y_r[st], in_=ysb)

            units = deque()

            def add_qk(w_sb, dst_sb, mt, c):
                state = {}
                units.append(lambda: qk_proj_half(w_sb, dst_sb, mt, c, True, state))
                units.append(lambda: qk_proj_half(w_sb, dst_sb, mt, c, False, state))

            def pop_unit():
                if units:
                    units.popleft()()

            def emit_pv(uv, mt, nkt, ex, kt, qlo):
                for parity in (0, 1):
                    head = 2 * mt + parity
                    blo, bhi = V_BLK[head]
                    nc.tensor.matmul(
                        uv[0 : bhi - blo, parity, qlo:512],
                        lhsT=v_sb[:, kt, blo:bhi],
                        rhs=ex[:, parity, qlo:512],
                        start=(kt == 0),
                        stop=(kt == nkt - 1),
                        skip_group_check=True,
                    )

            def normalize_a(uv, mt, qc):
                # Part A (DVE only, emitted right at block end): pull the
                # denominator rows out of U into SBUF bf16 rows for the
                # partition-broadcast matmul. reciprocal_approx_fast (custom
                # DVE ucode) only works on APs based at partition 0, so the
                # odd head (denom at partition 0) inverts here and broadcasts
                # 1/r; the even head (denom at partition 64) broadcasts raw r
                # and inverts after the broadcast.
                rr = small_pool.tile([128, 2, 512], BF16, tag="rr", name="rr")
                nc.vector.tensor_copy(rr[64:65, 0, :], uv[64:65, 0, :])
                ri = small_pool.tile([128, 512], F32, tag="ri", name="ri")
                nc.vector.reciprocal_approx_fast(out=ri[0:1, :], in_=uv[0:1, 1, :])
                nc.vector.tensor_copy(rr[0:1, 1, :], ri[0:1, :])
                return uv, rr

            def normalize_b(state, mt, qc):
                # Part B (a couple of iters into the next block): broadcast
                # along partitions via a ones-column matmul into PSUM, invert
                # the even head's r, and scale U into ctxT. Frees U ~2us after
                # the block's last PV instead of ~7us (DRAM-bounce dance).
                uv, rr = state
                psb0 = ps_p_pool.tile([128, 512], F32, tag="p", name="psb0")
                psb1 = ps_p_pool.tile([128, 512], F32, tag="p", name="psb1")
                nc.tensor.matmul(
                    psb0[0:64, :],
                    lhsT=ones_sb[64:65, :],
                    rhs=rr[64:65, 0, :],
                    start=True,
                    stop=True,
                )
                nc.tensor.matmul(
                    psb1[64:128, :],
                    lhsT=ones_sb[0:1, :],
                    rhs=rr[0:1, 1, :],
                    start=True,
                    stop=True,
                )
                rb = small_pool.tile([128, 512], F32, tag="rb", name="rb")
                nc.vector.reciprocal_approx_fast(out=rb[0:64, :], in_=psb0[0:64, :])
                # DVE reads at most one PSUM operand per op: the ctx multiply
                # reads U from PSUM, so the odd broadcast hops through SBUF.
                nc.vector.tensor_copy(rb[64:128, :], psb1[64:128, :])
                nc.vector.tensor_mul(
                    ctx_sb[qc][0:64, mt, :],
                    uv[0:64, 0, :],
                    rb[0:64, :],
                )
                nc.vector.tensor_mul(
                    ctx_sb[qc][64:128, mt, :],
                    uv[64:128, 1, :],
                    rb[64:128, :],
                )

            # ---- prelude: chunk-0 Q/K for head-pair 0 emitted directly ----
            st0 = {}
            qk_proj_half(wq_sb, qT_sb, 0, 0, True, st0)
            qk_proj_half(wq_sb, qT_sb, 0, 0, False, st0)
            st1 = {}
            qk_proj_half(wk_sb, kT_sb, 0, 0, True, st1)
            qk_proj_half(wk_sb, kT_sb, 0, 0, False, st1)

            # background FIFO (deadlines: proj chunk c fully done before its
            # attention/PV first touches it; O-proj chunk c queued once both
            # head-pairs' ctx for chunk c are final)
            add_qk(wq_sb, qT_sb, 1, 0)
            add_qk(wk_sb, kT_sb, 1, 0)
            for st in range(4):
                units.append(lambda st=st: v_proj(st))
            for mt in range(2):
                add_qk(wq_sb, qT_sb, mt, 1)
            for mt in range(2):
                add_qk(wk_sb, kT_sb, mt, 1)

            # ---- main attention loop, background units interleaved ----
            pending_b = None
            for qc in range(QC):
                # Each chunk's proj/O units enter the pop queue at the latest
                # block where their deadline still holds, keeping the deep
                # late blocks supplied with background PE work. O(c) may only
                # enter once chunk-c ctx is final (end of qc==c) — popping it
                # earlier deadlocks the PE queue behind its own ctx read.
                if qc == 1:
                    for st in range(4, 8):
                        units.append(lambda st=st: v_proj(st))
                    for mt in range(2):
                        add_qk(wq_sb, qT_sb, mt, 2)
                if qc == 2:
                    for mt in range(2):
                        add_qk(wk_sb, kT_sb, mt, 2)
                    for st in range(8, 12):
                        units.append(lambda st=st: v_proj(st))
                    for st in range(0, 8):
                        units.append(lambda st=st: o_proj_tile(st))
                    for mt in range(2):
                        add_qk(wq_sb, qT_sb, mt, 3)
                if qc == 3:
                    for mt in range(2):
                        add_qk(wk_sb, kT_sb, mt, 3)
                    for st in range(12, 16):
                        units.append(lambda st=st: v_proj(st))
                    for st in range(8, 12):
                        units.append(lambda st=st: o_proj_tile(st))
                nkt = 4 * (qc + 1)
                for mt in range(2):
                    # U is allocated lazily at the first PV so the ring's
                    # reuse dependency is recorded AFTER the deferred
                    # normalize_b reads of the previous block's U exist —
                    # allocating at block start lets the new block's start=True
                    # PV zero the bank while normalize_b still reads it.
                    ublock = []

                    def uview():
                        if not ublock:
                            u = ps_u_pool.tile([128, 1024], F32, tag="u", name="u")
                            ublock.append(u.rearrange("p (h q) -> p h q", q=512))
                        return ublock[0]

                    pv_q = []
                    for kt in range(nkt):
                        di = kt - 4 * qc
                        qlo = 128 * di if di >= 0 else 0
                        ps_s = ps_s_pool.tile([128, 1024], F32, tag="s", name="ps_s")
                        sv = ps_s.rearrange("p (h q) -> p h q", q=512)
                        for parity in (0, 1):
                            pofs = 64 * parity
                            nc.tensor.matmul(
                                sv[:, parity, qlo:512],
                                lhsT=kT_sb[
                                    pofs : pofs + 64, mt, 128 * kt : 128 * (kt + 1)
                                ],
                                rhs=qT_sb[
                                    pofs : pofs + 64,
                                    mt,
                                    512 * qc + qlo : 512 * (qc + 1),
                                ],
                                start=True,
                                stop=True,
                            )
                        ex = ex_pool.tile([128, 2, 512], BF16, tag="ex", name="ex")
                        nc.scalar.activation(
                            ex[:, :, qlo:512],
                            sv[:, :, qlo:512],
                            mybir.ActivationFunctionType.Exp,
                            scale=1.0 / np.sqrt(HD),
                        )
                        if di >= 0:
                            # gpsimd (Pool) is idle during attention and the
                            # mask is SBUF-only -> keep it off the busy DVE
                            nc.gpsimd.tensor_mul(
                                ex[:, :, qlo : qlo + 128],
                                ex[:, :, qlo : qlo + 128],
                                tri_sb,
                            )
                        pv_q.append((ex, kt, qlo))
                        # the deferred normalize_b must precede any popped
                        # unit (an O-proj unit may read the ctx it writes),
                        # and nothing pops at kt==0 so it's never outrun
                        if kt == 1 and pending_b is not None:
                            normalize_b(*pending_b)
                            pending_b = None
                        if kt > 0:
                            for _ in range(POPS[qc]):
                                pop_unit()
                        if len(pv_q) > LAG[qc]:
                            emit_pv(uview(), mt, nkt, *pv_q.pop(0))
                    while pv_q:
                        emit_pv(uview(), mt, nkt, *pv_q.pop(0))
                    pending_b = (normalize_a(uview(), mt, qc), mt, qc)

            # ---- drain: leftover units, then chunk-3 output projection ----
            normalize_b(*pending_b)
            while units:
                pop_unit()
            for st in range(12, 16):
                o_proj_tile(st)

            if debug:
                for c in range(QC):
                    for mt in range(2):
                        tmpc = io_pool.tile([128, 1024], F32, tag="dtmp", name="dtmpc")
                        nc.vector.tensor_copy(tmpc[:, 0:512], ctx_sb[c][:, mt, :])
                        nc.sync.dma_start(
                            out=dbg["d_ctxT"][:, 2048 * mt + 512 * c : 2048 * mt + 512 * (c + 1)],
                            in_=tmpc[:, 0:512],
                        )
                for nm, sb in (
                    ("d_qT", qT_sb),
                    ("d_kT", kT_sb),
                    ("d_v", v_sb),
                ):
                    flat = sb.rearrange("p a b -> p (a b)")
                    w = flat.shape[1]
                    for off in range(0, w, 512):
                        wid = min(512, w - off)
                        tmp2 = io_pool.tile([128, 1024], F32, tag="dtmp", name="dtmp")
                        nc.vector.tensor_copy(tmp2[:, 0:wid], flat[:, off : off + wid])
                        nc.sync.dma_start(
                            out=dbg[nm][:, off : off + wid], in_=tmp2[:, 0:wid]
                        )
    nc.finalize()
    return nc


_NC = None


def _get_nc():
    global _NC
    if _NC is None:
        _NC = _build_nc()
    return _NC


def kernel(x, Wq, Wk, Wv, Wo):
    x = np.asarray(x, dtype=np.float32)
    bf = ml_dtypes.bfloat16
    in_maps = []
    for c in range(NCORES):
        b, g = divmod(c, 4)
        sl = slice(g * DH, (g + 1) * DH)
        in_maps.append(
            {
                "xT": np.ascontiguousarray(x[b].T).astype(bf),
                "wq": np.ascontiguousarray(np.asarray(Wq)[:, sl]).astype(bf),
                "wk": np.ascontiguousarray(np.asarray(Wk)[:, sl]).astype(bf),
                "wv": np.ascontiguousarray(np.asarray(Wv)[:, sl]).astype(bf),
                "wo": np.ascontiguousarray(np.asarray(Wo)[sl, :]).astype(bf),
            }
        )
    global _last_in_maps
    _last_in_maps = in_maps
    res = run_bass_kernel_spmd(
        _get_nc(), in_maps, core_ids=list(range(NCORES)), trace=False
    )
    out = np.zeros((B, N, D), dtype=np.float32)
    for c in range(NCORES):
        out[c // 4] += res.results[c]["y"]
    return out


# revision 34
# speedup vs baseline: 1.2379x; 1.0185x over previous
"""Multi-head causal attention (B=2, N=2048, D=1024, H=16) on 8 TRN2 NeuronCores.

Sharding: data-parallel over batch (2) x tensor-parallel over head groups (4),
so each core handles one batch element and 4 heads (256 of the 1024 hidden
channels). Wq/Wk/Wv are column-sharded, Wo row-sharded; each core emits a
partial output [2048, 1024] that the host sums over the 4 head groups.

Single fully-interleaved schedule (all matmuls bf16, fp32 PSUM):
  - The QKV projections, attention (S = K^T Q -> exp -> PV), and output
    projection are woven into ONE instruction stream per engine so the
    Act-engine exp work (~70us) hides under projection matmuls and the PE
    never idles between phases. Background work (next seq-chunk's Q/K/V
    projections, previous chunk's output projection) is kept in a FIFO of
    small emission units popped between attention k-tile groups.
  - Causal q-restriction: for diagonal-crossing k-tiles only the q-range
    that can be unmasked is computed (S matmul rhs, exp, and PV are all
    restricted), and masking is a single [128,2,128] additive triangle
    (-256 on masked elements, applied to PSUM scores before the exp's 1/8
    scale) instead of full-tile 0/1 multiplies.
  - Layouts: head pairs packed per mt (even head partitions 0-63, odd
    64-127); S^T tiles [128k, 2x512q]; V stored per seq-tile with ones
    columns so the PV matmul accumulates the softmax denominator (even
    head: denom at U partition 64; odd head: denom at partition 0 with a
    zero strip keeping partitions 1-63 inert).
  - PSUM budget (8 banks): S ring 2x[128,1024], U accumulator 1x[128,1024]
    (PV lags exp by LAG k-tiles so the one U buffer is free in time),
    projection ring 2x[128,512].
  - Softmax normalization: denominator row -> reciprocal (DVE ucode, only
    legal at partition 0) -> partition-broadcast via a DRAM bounce on the
    gpsimd DMA queue (stride-0 partition APs are only legal for DRAM
    sources) -> ctx multiply. All off the PE queue.
"""

import sys

sys.path.insert(0, "/opt/trn_rl_repo")

from collections import deque

import numpy as np
import ml_dtypes

import concourse.bass as bass
import concourse.bacc as bacc
import concourse.mybir as mybir
from concourse.tile import TileContext
from concourse.bass_utils import run_bass_kernel_spmd

BF16 = mybir.dt.bfloat16
F32 = mybir.dt.float32

B, N, D, H = 2, 2048, 1024, 16
HD = 64          # head dim
HPC = 4          # heads per core
DH = HPC * HD    # 256 hidden channels per core
NCORES = 8
KT = D // 128    # 8 contraction tiles over D
ST = N // 128    # 16 seq tiles
QC = N // 512    # 4 q-chunks of 512

LAG = {0: 4, 1: 4, 2: 6, 3: 6}   # PV trails exp by this many k-tiles
POPS = {0: 4, 1: 2, 2: 2, 3: 2}  # background units popped per k-tile iter

# v_sb per-seq-tile column layout: for each head pair, an "even" block
# [V(64) | ones(1)] (matmul M=65 -> U partitions 0..64, denom at 64) and an
# "odd" block [ones(1) | zeros(63) | V(64)] (M=128 -> U partitions 64..127
# hold data, denom at partition 0, zeros keep partitions 1..63 inert).
V_BLK = {0: (0, 65), 1: (65, 193), 2: (193, 258), 3: (258, 386)}
V_COLS = 386


def _build_nc(debug: bool = False) -> bass.Bass:
    nc = bacc.Bacc()
    xT = nc.declare_dram_parameter("xT", [D, N], BF16, isOutput=False)
    wq = nc.declare_dram_parameter("wq", [D, DH], BF16, isOutput=False)
    wk = nc.declare_dram_parameter("wk", [D, DH], BF16, isOutput=False)
    wv = nc.declare_dram_parameter("wv", [D, DH], BF16, isOutput=False)
    wo = nc.declare_dram_parameter("wo", [DH, D], BF16, isOutput=False)
    y = nc.declare_dram_parameter("y", [N, D], BF16, isOutput=True)
    if debug:
        dbg = {
            "d_qT": nc.declare_dram_parameter("d_qT", [128, 2 * N], F32, isOutput=True),
            "d_kT": nc.declare_dram_parameter("d_kT", [128, 2 * N], F32, isOutput=True),
            "d_v": nc.declare_dram_parameter("d_v", [128, ST * V_COLS], F32, isOutput=True),
            "d_ctxT": nc.declare_dram_parameter("d_ctxT", [128, 2 * N], F32, isOutput=True),
        }

    xT_r = xT.rearrange("(t p) n -> t p n", p=128)
    wq_r = wq.rearrange("(t p) m -> t p m", p=128)
    wk_r = wk.rearrange("(t p) m -> t p m", p=128)
    wv_r = wv.rearrange("(t p) m -> t p m", p=128)
    wo_r = wo.rearrange("(t p) m -> t p m", p=128)
    y_r = y.rearrange("(t p) m -> t p m", p=128)

    with TileContext(nc) as tc:
        with (
            tc.tile_pool(name="const", bufs=1) as cpool,
            tc.tile_pool(name="io", bufs=3) as io_pool,
            tc.tile_pool(name="exps", bufs=8) as ex_pool,
            tc.tile_pool(name="small", bufs=4) as small_pool,
            tc.tile_pool(name="ps_s", bufs=2, space="PSUM") as ps_s_pool,
            tc.tile_pool(name="ps_u", bufs=1, space="PSUM") as ps_u_pool,
            tc.tile_pool(name="ps_p", bufs=2, space="PSUM") as ps_p_pool,
        ):
            xT_sb = cpool.tile([128, KT, N], BF16)
            wq_sb = cpool.tile([128, KT, DH], BF16)
            wk_sb = cpool.tile([128, KT, DH], BF16)
            wv_sb = cpool.tile([128, KT, DH], BF16)
            wo_sb = cpool.tile([128, 2, D], BF16)
            qT_sb = cpool.tile([128, 2, N], BF16)
            kT_sb = cpool.tile([128, 2, N], BF16)
            v_sb = cpool.tile([128, ST, V_COLS], BF16)
            # per-chunk ctx tiles: a single [128, 2, N] tile accumulates so
            # many distinct access regions that the subtile dependency
            # tracker misses write->read edges (observed as a nondeterministic
            # race: O-proj matmuls reading ctx before the normalize multiply).
            ctx_sb = [cpool.tile([128, 2, 512], BF16, name=f"ctx{c}") for c in range(QC)]
            tri_sb = cpool.tile([128, 2, 128], BF16)
            ones_sb = cpool.tile([128, 64], BF16)

            # ---- input DMAs, in consumption order ----
            # sync queue: wq + chunk-0 x columns (gate the first matmuls),
            # then wk/wv and chunk 1; Activation hwdge queue (otherwise idle):
            # chunks 2-3 and wo, halving the serial input-load time.
            xT_c = xT.rearrange("(t p) (c n) -> c p t n", p=128, n=512)
            xc_sb = xT_sb.rearrange("p t (c n) -> c p t n", n=512)
            nc.scalar.dma_start(out=wq_sb, in_=wq.rearrange("(t p) m -> p t m", p=128))
            # chunk-0 in four kt-range pieces so the first projection
            # accumulation (kt 0-3) starts ~2us earlier than one 1MB DMA
            for t4 in range(4):
                nc.sync.dma_start(
                    out=xc_sb[0][:, 2 * t4 : 2 * t4 + 2, :],
                    in_=xT_c[0][:, 2 * t4 : 2 * t4 + 2, :],
                )
            nc.scalar.dma_start(out=wk_sb, in_=wk.rearrange("(t p) m -> p t m", p=128))
            nc.scalar.dma_start(out=wv_sb, in_=wv.rearrange("(t p) m -> p t m", p=128))
            nc.sync.dma_start(out=xc_sb[1], in_=xT_c[1])
            nc.scalar.dma_start(out=xc_sb[2], in_=xT_c[2])
            nc.sync.dma_start(out=xc_sb[3], in_=xT_c[3])
            nc.scalar.dma_start(
                out=wo_sb, in_=wo.rearrange("(t p) m -> p t m", p=128)
            )

            # Causal triangle for the q-block crossing each diagonal k-tile:
            # 1.0 where q >= k (keep), 0.0 masked; multiplies exp's output so
            # the S->exp chain has no DVE hop (the exp->PV path has LAG slack).
            nc.vector.memset(tri_sb, 1.0)
            nc.gpsimd.affine_select(
                out=tri_sb,
                in_=tri_sb,
                compare_op=mybir.AluOpType.is_ge,
                fill=0.0,
                base=0,
                pattern=[[0, 2], [1, 128]],
                channel_multiplier=-1,
            )
            nc.vector.memset(ones_sb, 1.0)

            # ones / zeros scaffolding of the V blocks (all seq tiles at once)
            nc.vector.memset(v_sb[:, :, 66:129], 0.0)
            nc.vector.memset(v_sb[:, :, 259:322], 0.0)
            for col in (64, 65, 257, 258):
                nc.vector.memset(v_sb[:, :, col : col + 1], 1.0)

            # ---- emission helpers ----
            def qk_proj_half(w_sb, dst_sb, mt, c, lo_half, state):
                # half a [128, 512] projection accumulation group (4 of 8 kt)
                if lo_half:
                    state["ps"] = ps_p_pool.tile(
                        [128, 512], F32, tag="p", name="ps_qk"
                    )
                ps = state["ps"]
                for kt in range(0 if lo_half else 4, 4 if lo_half else 8):
                    nc.tensor.matmul(
                        ps,
                        lhsT=w_sb[:, kt, 128 * mt : 128 * (mt + 1)],
                        rhs=xT_sb[:, kt, 512 * c : 512 * (c + 1)],
                        start=(kt == 0),
                        stop=(kt == KT - 1),
                    )
                if not lo_half:
                    nc.vector.tensor_copy(
                        dst_sb[:, mt, 512 * c : 512 * (c + 1)], ps
                    )

            def v_proj(st):
                ps = ps_p_pool.tile([128, 512], F32, tag="p", name="ps_v")
                psv = ps[:, 0:DH]
                for kt in range(KT):
                    nc.tensor.matmul(
                        psv,
                        lhsT=xT_sb[:, kt, 128 * st : 128 * (st + 1)],
                        rhs=wv_sb[:, kt, :],
                        start=(kt == 0),
                        stop=(kt == KT - 1),
                    )
                ps_h = psv.rearrange("p (h d) -> p h d", d=HD)
                # even heads 0,2 -> v col offsets 0,193; odd heads 1,3 -> 129,322
                ev = bass.AP(
                    tensor=v_sb.tensor,
                    offset=v_sb[:, st, 0:1].offset,
                    ap=[v_sb.ap[0], [193, 2], [1, HD]],
                )
                od = bass.AP(
                    tensor=v_sb.tensor,
                    offset=v_sb[:, st, 129:130].offset,
                    ap=[v_sb.ap[0], [193, 2], [1, HD]],
                )
                in_ev = bass.AP(
                    tensor=ps.tensor,
                    offset=ps_h[:, 0, :].offset,
                    ap=[ps.ap[0], [2 * HD, 2], [1, HD]],
                )
                in_od = bass.AP(
                    tensor=ps.tensor,
                    offset=ps_h[:, 1, :].offset,
                    ap=[ps.ap[0], [2 * HD, 2], [1, HD]],
                )
                nc.vector.tensor_copy(ev, in_ev)
                nc.vector.tensor_copy(od, in_od)

            def o_proj_tile(st):
                # kt2-outer so each ctx weight tile loads once for both
                # output halves (halves the LDWEIGHTS count)
                pss = [
                    ps_p_pool.tile([128, 512], F32, tag="p", name="ps_o")
                    for _ in range(2)
                ]
                for kt2 in range(2):
                    for half in range(2):
                        nc.tensor.matmul(
                            pss[half],
                            lhsT=ctx_sb[st // 4][
                                :, kt2, 128 * (st % 4) : 128 * (st % 4 + 1)
                            ],
                            rhs=wo_sb[:, kt2, 512 * half : 512 * (half + 1)],
                            start=(kt2 == 0),
                            stop=(kt2 == 1),
                        )
                ysb = io_pool.tile([128, 1024], BF16, tag="y", name="ysb")
                for half in range(2):
                    nc.vector.tensor_copy(
                        ysb[:, 512 * half : 512 * (half + 1)], pss[half]
                    )
                nc.sync.dma_start(out=y_r[st], in_=ysb)

            units = deque()

            def add_qk(w_sb, dst_sb, mt, c):
                state = {}
                units.append(lambda: qk_proj_half(w_sb, dst_sb, mt, c, True, state))
                units.append(lambda: qk_proj_half(w_sb, dst_sb, mt, c, False, state))

            def pop_unit():
                if units:
                    units.popleft()()

            def emit_pv(uv, mt, nkt, ex, kt, qlo):
                for parity in (0, 1):
                    head = 2 * mt + parity
                    blo, bhi = V_BLK[head]
                    nc.tensor.matmul(
                        uv[0 : bhi - blo, parity, qlo:512],
                        lhsT=v_sb[:, kt, blo:bhi],
                        rhs=ex[:, parity, qlo:512],
                        start=(kt == 0),
                        stop=(kt == nkt - 1),
                        skip_group_check=True,
                    )

            def normalize_a(uv, mt, qc):
                # Part A (DVE only, emitted right at block end): pull the
                # denominator rows out of U into SBUF bf16 rows for the
                # partition-broadcast matmul. reciprocal_approx_fast (custom
                # DVE ucode) only works on APs based at partition 0, so the
                # odd head (denom at partition 0) inverts here and broadcasts
                # 1/r; the even head (denom at partition 64) broadcasts raw r
                # and inverts after the broadcast.
                rr = small_pool.tile([128, 2, 512], BF16, tag="rr", name="rr")
                nc.vector.tensor_copy(rr[64:65, 0, :], uv[64:65, 0, :])
                ri = small_pool.tile([128, 512], F32, tag="ri", name="ri")
                nc.vector.reciprocal_approx_fast(out=ri[0:1, :], in_=uv[0:1, 1, :])
                nc.vector.tensor_copy(rr[0:1, 1, :], ri[0:1, :])
                return uv, rr

            def normalize_b(state, mt, qc):
                # Part B (a couple of iters into the next block): broadcast
                # along partitions via a ones-column matmul into PSUM, invert
                # the even head's r, and scale U into ctxT. Frees U ~2us after
                # the block's last PV instead of ~7us (DRAM-bounce dance).
                uv, rr = state
                psb0 = ps_p_pool.tile([128, 512], F32, tag="p", name="psb0")
                psb1 = ps_p_pool.tile([128, 512], F32, tag="p", name="psb1")
                nc.tensor.matmul(
                    psb0[0:64, :],
                    lhsT=ones_sb[64:65, :],
                    rhs=rr[64:65, 0, :],
                    start=True,
                    stop=True,
                )
                nc.tensor.matmul(
                    psb1[64:128, :],
                    lhsT=ones_sb[0:1, :],
                    rhs=rr[0:1, 1, :],
                    start=True,
                    stop=True,
                )
                rb = small_pool.tile([128, 512], F32, tag="rb", name="rb")
                nc.vector.reciprocal_approx_fast(out=rb[0:64, :], in_=psb0[0:64, :])
                # DVE reads at most one PSUM operand per op: the ctx multiply
                # reads U from PSUM, so the odd broadcast hops through SBUF.
                nc.vector.tensor_copy(rb[64:128, :], psb1[64:128, :])
                nc.vector.tensor_mul(
                    ctx_sb[qc][0:64, mt, :],
                    uv[0:64, 0, :],
                    rb[0:64, :],
                )
                nc.vector.tensor_mul(
                    ctx_sb[qc][64:128, mt, :],
                    uv[64:128, 1, :],
                    rb[64:128, :],
                )

            # ---- prelude: chunk-0 Q/K for head-pair 0 emitted directly ----
            st0 = {}
            qk_proj_half(wq_sb, qT_sb, 0, 0, True, st0)
            qk_proj_half(wq_sb, qT_sb, 0, 0, False, st0)
            st1 = {}
            qk_proj_half(wk_sb, kT_sb, 0, 0, True, st1)
            qk_proj_half(wk_sb, kT_sb, 0, 0, False, st1)

            # background FIFO (deadlines: proj chunk c fully done before its
            # attention/PV first touches it; O-proj chunk c queued once both
            # head-pairs' ctx for chunk c are final)
            add_qk(wq_sb, qT_sb, 1, 0)
            add_qk(wk_sb, kT_sb, 1, 0)
            for st in range(4):
                units.append(lambda st=st: v_proj(st))
            for mt in range(2):
                add_qk(wq_sb, qT_sb, mt, 1)
            for mt in range(2):
                add_qk(wk_sb, kT_sb, mt, 1)

            # ---- main attention loop, background units interleaved ----
            pending_b = None
            for qc in range(QC):
                # Each chunk's proj/O units enter the pop queue at the latest
                # block where their deadline still holds, keeping the deep
                # late blocks supplied with background PE work. O(c) may only
                # enter once chunk-c ctx is final (end of qc==c) — popping it
                # earlier deadlocks the PE queue behind its own ctx read.
                if qc == 1:
                    for st in range(4, 8):
                        units.append(lambda st=st: v_proj(st))
                    for mt in range(2):
                        add_qk(wq_sb, qT_sb, mt, 2)
                if qc == 2:
                    for mt in range(2):
                        add_qk(wk_sb, kT_sb, mt, 2)
                    for st in range(8, 12):
                        units.append(lambda st=st: v_proj(st))
                    for st in range(0, 8):
                        units.append(lambda st=st: o_proj_tile(st))
                    for mt in range(2):
                        add_qk(wq_sb, qT_sb, mt, 3)
                if qc == 3:
                    for mt in range(2):
                        add_qk(wk_sb, kT_sb, mt, 3)
                    for st in range(12, 16):
                        units.append(lambda st=st: v_proj(st))
                    for st in range(8, 12):
                        units.append(lambda st=st: o_proj_tile(st))
                nkt = 4 * (qc + 1)
                for mt in range(2):
                    # U is allocated lazily at the first PV so the ring's
                    # reuse dependency is recorded AFTER the deferred
                    # normalize_b reads of the previous block's U exist —
                    # allocating at block start lets the new block's start=True
                    # PV zero the bank while normalize_b still reads it.
                    ublock = []

                    def uview():
                        if not ublock:
                            u = ps_u_pool.tile([128, 1024], F32, tag="u", name="u")
                            ublock.append(u.rearrange("p (h q) -> p h q", q=512))
                        return ublock[0]

                    pv_q = []
                    for kt in range(nkt):
                        di = kt - 4 * qc
                        qlo = 128 * di if di >= 0 else 0
                        ps_s = ps_s_pool.tile([128, 1024], F32, tag="s", name="ps_s")
                        sv = ps_s.rearrange("p (h q) -> p h q", q=512)
                        for parity in (0, 1):
                            pofs = 64 * parity
                            nc.tensor.matmul(
                                sv[:, parity, qlo:512],
                                lhsT=kT_sb[
                                    pofs : pofs + 64, mt, 128 * kt : 128 * (kt + 1)
                                ],
                                rhs=qT_sb[
                                    pofs : pofs + 64,
                                    mt,
                                    512 * qc + qlo : 512 * (qc + 1),
                                ],
                                start=True,
                                stop=True,
                            )
                        ex = ex_pool.tile([128, 2, 512], BF16, tag="ex", name="ex")
                        nc.scalar.activation(
                            ex[:, :, qlo:512],
                            sv[:, :, qlo:512],
                            mybir.ActivationFunctionType.Exp,
                            scale=1.0 / np.sqrt(HD),
                        )
                        if di >= 0:
                            # gpsimd (Pool) is idle during attention and the
                            # mask is SBUF-only -> keep it off the busy DVE
                            nc.gpsimd.tensor_mul(
                                ex[:, :, qlo : qlo + 128],
                                ex[:, :, qlo : qlo + 128],
                                tri_sb,
                            )
                        pv_q.append((ex, kt, qlo))
                        # the deferred normalize_b must precede any popped
                        # unit (an O-proj unit may read the ctx it writes),
                        # and nothing pops at kt==0 so it's never outrun
                        if kt == 1 and pending_b is not None:
                            normalize_b(*pending_b)
                            pending_b = None
                        if kt > 0:
                            for _ in range(POPS[qc]):
                                pop_unit()
                        if len(pv_q) > LAG[qc]:
                            emit_pv(uview(), mt, nkt, *pv_q.pop(0))
                    while pv_q:
                        emit_pv(uview(), mt, nkt, *pv_q.pop(0))
                    pending_b = (normalize_a(uview(), mt, qc), mt, qc)

            # ---- drain: leftover units, then chunk-3 output projection ----
            normalize_b(*pending_b)
            while units:
                pop_unit()
            for st in range(12, 16):
                o_proj_tile(st)

            if debug:
                for c in range(QC):
                    for mt in range(2):
                        tmpc = io_pool.tile([128, 1024], F32, tag="dtmp", name="dtmpc")
                        nc.vector.tensor_copy(tmpc[:, 0:512], ctx_sb[c][:, mt, :])
                        nc.sync.dma_start(
                            out=dbg["d_ctxT"][:, 2048 * mt + 512 * c : 2048 * mt + 512 * (c + 1)],
                            in_=tmpc[:, 0:512],
                        )
                for nm, sb in (
                    ("d_qT", qT_sb),
                    ("d_kT", kT_sb),
                    ("d_v", v_sb),
                ):
                    flat = sb.rearrange("p a b -> p (a b)")
                    w = flat.shape[1]
                    for off in range(0, w, 512):
                        wid = min(512, w - off)
                        tmp2 = io_pool.tile([128, 1024], F32, tag="dtmp", name="dtmp")
                        nc.vector.tensor_copy(tmp2[:, 0:wid], flat[:, off : off + wid])
                        nc.sync.dma_start(
                            out=dbg[nm][:, off : off + wid], in_=tmp2[:, 0:wid]
                        )
    nc.finalize()
    return nc


_NC = None


def _get_nc():
    global _NC
    if _NC is None:
        _NC = _build_nc()
    return _NC


def kernel(x, Wq, Wk, Wv, Wo):
    x = np.asarray(x, dtype=np.float32)
    bf = ml_dtypes.bfloat16
    in_maps = []
    for c in range(NCORES):
        b, g = divmod(c, 4)
        sl = slice(g * DH, (g + 1) * DH)
        in_maps.append(
            {
                "xT": np.ascontiguousarray(x[b].T).astype(bf),
                "wq": np.ascontiguousarray(np.asarray(Wq)[:, sl]).astype(bf),
                "wk": np.ascontiguousarray(np.asarray(Wk)[:, sl]).astype(bf),
                "wv": np.ascontiguousarray(np.asarray(Wv)[:, sl]).astype(bf),
                "wo": np.ascontiguousarray(np.asarray(Wo)[sl, :]).astype(bf),
            }
        )
    global _last_in_maps
    _last_in_maps = in_maps
    res = run_bass_kernel_spmd(
        _get_nc(), in_maps, core_ids=list(range(NCORES)), trace=False
    )
    out = np.zeros((B, N, D), dtype=np.float32)
    for c in range(NCORES):
        out[c // 4] += res.results[c]["y"]
    return out
